# revision 12
# baseline (speedup 1.0000x reference)
"""AttentionAggregator kernel for 8 Trainium2 NeuronCores — v5d.

Trace analysis of v4 (235us): the critical path was SWDGE descriptor
generation on the Q7 cores — serial at ~2ns/descriptor across all 4
queues (88064 descs/core = 174us), with DMA drains overlapped only one
round deep (engines idle ~28%, PE ~17% busy).

v5 halves the descriptor count by gathering 512B per descriptor (two
feature rows). A per-core permutation of the feature table (built on
host, all 500000 rows present exactly once) places pairs of rows needed
by the same target-group adjacently, so ~94% of descriptors carry two
useful rows. Duplicate uses of a row by other groups fetch the row's
containing pair and use only one half (wt zeros on the other half).
Now drain-bound: ~92us/engine gather (512B packets at ~31ns: ~14ns
fixed + bytes/30GBps) + ~40us/engine HWDGE (wt/idx/out).

Layout per core:
  - table: featp [250000 pairs x 512B bf16], host-permuted.
  - 8 index windows of 31250 pairs (int16 idx range).
  - column = (group, window): 128 pair-slots sorted by idx (DRAM-
    friendly ascending addresses), one gather column.
  - per column TWO matmuls (even-half rows, odd-half rows) with separate
    [128 x 64] weight blocks; PSUM accumulation unchanged from v4.
  - calls: window 0 in sixths (early doorbells — drains for a call only
    start at its gen end), windows 1-6 halves, window 7 quarters.
  - output in bf16 (rel err 6.0e-3 vs 3.5e-3 fp32; gate is 2e-2).

Measured 161-175us across runs (device variance ±5%); v4 was 235us.
A/B-tested dead ends: single_packet=True (device crash), 1 SWDGE queue
(2.4x worse), warmup gathers (slower), per-call idx chunking (18us
startup from serial HWDGE issue).
"""

import sys

sys.path.insert(0, "/opt/trn_rl_repo")

import numpy as np
import ml_dtypes

N_NODES = 500000
D = 128
K = 32
B = 20000
NCORES = 8
BPC = B // NCORES
NW = 8
PAIRS = N_NODES // 2
WPAIRS = PAIRS // NW           # 31250 pair indices per window (< 32768)
GCAP = 64
NGRP_TRY = (43, 44, 45, 46)

bf16 = ml_dtypes.bfloat16


def _prepare_core(nb, wgt, ngrp, salt):
    """Build per-core permutation + idx16 + wt. Returns (tensors, row_map)
    or (None, None) if the window packing is infeasible at this ngrp.
    `salt` varies the primary-group hash; some salts balance groups
    enough to pack at a smaller ngrp."""
    ncols = NW * ngrp

    # --- targets -> groups (balanced contiguous chunks) ---
    sizes = np.full(ngrp, BPC // ngrp, np.int64)
    sizes[: BPC % ngrp] += 1
    if sizes.max() > GCAP:
        return None, None
    gof_t = np.repeat(np.arange(ngrp), sizes)
    gstart = np.concatenate(([0], np.cumsum(sizes)))
    rid_t = np.arange(BPC) - gstart[gof_t]
    row_map = np.full(ngrp * GCAP, -1, np.int64)
    row_map[gof_t * GCAP + rid_t] = np.arange(BPC)

    # --- uses, dedup to (group,row) needs ---
    t_e = np.repeat(np.arange(BPC), K)
    r_e = nb.reshape(-1).astype(np.int64)
    w_e = wgt.reshape(-1).astype(np.float32)
    g_e = gof_t[t_e]
    key = g_e * N_NODES + r_e
    uorder = np.argsort(key, kind="stable")
    ko = key[uorder]
    new = np.ones(ko.size, bool)
    new[1:] = ko[1:] != ko[:-1]
    starts = np.nonzero(new)[0]
    need_g = ko[starts] // N_NODES
    need_r = ko[starts] % N_NODES
    nneed = starts.size
    use_need = np.cumsum(new) - 1          # sorted-use -> need id

    # --- primary need per unique row (pseudo-random group: balances
    #     primary/secondary load across groups) ---
    ord_rg = np.lexsort((need_g, need_r))
    rr = need_r[ord_rg]
    firstmask = np.ones(nneed, bool)
    firstmask[1:] = rr[1:] != rr[:-1]
    rstart = np.nonzero(firstmask)[0]
    rcnt = np.diff(np.append(rstart, nneed))
    pick = rstart + ((need_r[ord_rg[rstart]] * 2654435761 >> salt) % rcnt)
    prim_ids = ord_rg[pick]                # one per unique row, rows ascending
    prim_rows = need_r[prim_ids]
    is_prim = np.zeros(nneed, bool)
    is_prim[prim_ids] = True
    sec_ids = np.nonzero(~is_prim)[0]
    sec_prim = prim_ids[np.searchsorted(prim_rows, need_r[sec_ids])]

    # --- pair up primaries per group ---
    pair_g_l, pairA_l, pairB_l = [], [], []
    for g in range(ngrp):
        pn = prim_ids[need_g[prim_ids] == g]
        if pn.size % 2:
            pn = np.append(pn, -1)
        pairA_l.append(pn[0::2])
        pairB_l.append(pn[1::2])
        pair_g_l.append(np.full(pn.size // 2, g, np.int64))
    pair_g = np.concatenate(pair_g_l)
    pairA = np.concatenate(pairA_l)
    pairB = np.concatenate(pairB_l)
    P = pair_g.size
    vB = pairB >= 0

    # need -> (pair, half)
    need_pair = np.full(nneed, -1, np.int64)
    need_half = np.zeros(nneed, np.int64)
    need_pair[pairA] = np.arange(P)
    need_half[pairA] = 0
    need_pair[pairB[vB]] = np.arange(P)[vB]
    need_half[pairB[vB]] = 1
    sec_pair = need_pair[sec_prim]
    sec_half = need_half[sec_prim]
    sec_g = need_g[sec_ids]

    # --- window assignment: even spread per group, fix overflows ---
    pw = np.empty(P, np.int64)
    for g in range(ngrp):
        idx = np.nonzero(pair_g == g)[0]
        pw[idx] = (np.arange(idx.size, dtype=np.int64) * NW) // max(idx.size, 1)

    pair_has_sec = np.zeros(P, bool)
    pair_has_sec[sec_pair] = True
    ok = False
    for _ in range(100):
        cnt = np.zeros((ngrp, NW), np.int64)
        np.add.at(cnt, (pair_g, pw), 1)
        np.add.at(cnt, (sec_g, pw[sec_pair]), 1)
        over = np.argwhere(cnt > 128)
        if over.size == 0:
            ok = True
            break
        progress = False
        for g, w in over:
            excess = cnt[g, w] - 128
            cand = np.nonzero((pair_g == g) & (pw == w))[0]
            # prefer moving pairs with no secondary refs (no ripple)
            cand = cand[np.argsort(pair_has_sec[cand], kind="stable")]
            slack_ws = np.argsort(cnt[g], kind="stable")
            ci = 0
            for wd in slack_ws:
                if wd == w:
                    continue
                room = 128 - cnt[g, wd]
                while room > 0 and excess > 0 and ci < cand.size:
                    pw[cand[ci]] = wd
                    ci += 1
                    room -= 1
                    excess -= 1
                    cnt[g, wd] += 1
                    cnt[g, w] -= 1
                    progress = True
                if excess <= 0:
                    break
        if not progress:
            break
    if not ok:
        return None, None

    # --- pair positions: spread within each window ---
    pair_pos = np.empty(P, np.int64)
    for w in range(NW):
        ids = np.nonzero(pw == w)[0]
        npw = ids.size
        if npw > WPAIRS:
            return None, None
        pos = (np.arange(npw, dtype=np.int64) * WPAIRS) // max(npw, 1)
        pair_pos[ids] = w * WPAIRS + pos

    # --- permutation (all rows exactly once) ---
    perm = np.full(N_NODES, -1, np.int64)
    rA = need_r[pairA]
    perm[2 * pair_pos] = rA
    rB = need_r[pairB[vB]]
    perm[2 * pair_pos[vB] + 1] = rB
    usedrows = np.zeros(N_NODES, bool)
    usedrows[rA] = True
    usedrows[rB] = True
    holes = np.nonzero(perm < 0)[0]
    rest = np.nonzero(~usedrows)[0]
    perm[holes] = rest

    # --- slots per column: pairs first, then secondaries ---
    col_of_pair = pw * ngrp + pair_g
    col_of_sec = pw[sec_pair] * ngrp + sec_g
    ecol = np.concatenate([col_of_pair, col_of_sec])
    eidx = np.concatenate([pair_pos % WPAIRS, pair_pos[sec_pair] % WPAIRS])
    # sort by (column, idx): ascending addresses within each column give
    # the SDMA engines DRAM-friendly access order
    eorder = np.lexsort((eidx, ecol))
    colcnt = np.bincount(ecol, minlength=ncols)
    assert colcnt.max() <= 128
    cstart = np.zeros(ncols, np.int64)
    cstart[1:] = np.cumsum(colcnt)[:-1]
    E = ecol.size
    rank = np.arange(E) - cstart[ecol[eorder]]
    slot_of_entry = np.empty(E, np.int64)
    slot_of_entry[eorder] = ecol[eorder] * 128 + rank

    idx_flat = np.zeros(ncols * 128, np.int16)
    idx_flat[slot_of_entry] = eidx.astype(np.int16)

    # --- weights ---
    need_slot = np.empty(nneed, np.int64)
    need_hf = np.empty(nneed, np.int64)
    need_slot[pairA] = slot_of_entry[:P]
    need_hf[pairA] = 0
    need_slot[pairB[vB]] = slot_of_entry[:P][vB]
    need_hf[pairB[vB]] = 1
    need_slot[sec_ids] = slot_of_entry[P:]
    need_hf[sec_ids] = sec_half

    un = use_need
    us = need_slot[un]
    uh = need_hf[un]
    ut = t_e[uorder]
    uw = w_e[uorder]
    urid = rid_t[ut]
    ucol = us // 128
    urow = us % 128
    wt = np.zeros((128, ncols * 2 * GCAP), np.float32)
    np.add.at(wt, (urow, (ucol * 2 + uh) * GCAP + urid), uw)

    nslot = ncols * 128
    wi = nslot // 16
    wrapped = idx_flat.reshape(wi, 16).T
    idx16 = np.tile(wrapped, (8, 1)).astype(np.int16)
    return dict(idx16=idx16, wt=wt.astype(bf16), perm=perm), row_map


_CACHE = {}


def _build_program(ngrp):
    import concourse.bacc as bacc
    import concourse.mybir as mybir
    import concourse.tile as tile

    ncols = NW * ngrp
    outrows = ngrp * GCAP
    nbank = (ngrp + 7) // 8
    CS = ngrp
    CH = [(CS + 1) // 2, CS - (CS + 1) // 2]

    nc = bacc.Bacc("TRN2", target_bir_lowering=False, debug=False,
                   num_devices=NCORES, num_swdge_queues=4,
                   dynamic_dma_scratch_size=32768)
    dt = mybir.dt
    featp_d = nc.dram_tensor("featp", [PAIRS, 2 * D], dt.bfloat16, kind="ExternalInput")
    idx_d = nc.dram_tensor("idx16", [128, ncols * 8], dt.int16, kind="ExternalInput")
    wt_d = nc.dram_tensor("wt", [128, ncols * 2 * GCAP], dt.bfloat16, kind="ExternalInput")
    wo_d = nc.dram_tensor("Wout", [128, D], dt.bfloat16, kind="ExternalInput")
    # bank-major contiguous output: row (bank*128+p) holds targets
    # bank*512 + j*128 + p for j in 0..nj-1 (host de-interleaves).
    # 128 descs of 1KB per bank instead of 512 descs of 256B.
    o_d = nc.dram_tensor("o", [nbank * 128, 4 * D], dt.bfloat16,
                         kind="ExternalOutput")

    # call list: window 0 in sixths (early doorbells -> drains start
    # ~20us sooner), windows 1-6 in halves, window 7 in quarters
    # (shorter final drains). Each entry: (col_start, n_cols).
    calls = []
    n6 = (CS + 5) // 6
    off = 0
    while off < CS:
        take = min(n6, CS - off)
        calls.append((off, take))
        off += take
    for w in range(1, NW - 1):
        calls.append((w * CS, CH[0]))
        calls.append((w * CS + CH[0], CH[1]))
    q4 = [(CS + 3) // 4] * 3
    q4.append(CS - sum(q4))
    off = (NW - 1) * CS
    for q in q4:
        calls.append((off, q))
        off += q

    with tile.TileContext(nc) as tc:
        with (
            tc.tile_pool(name="big", bufs=8) as big,
            tc.tile_pool(name="wtp", bufs=6) as wtp,
            tc.tile_pool(name="persist", bufs=1) as persist,
            tc.tile_pool(name="psum", bufs=1, space="PSUM") as psump,
            tc.tile_pool(name="epi", bufs=3) as epi,
            tc.tile_pool(name="episum", bufs=2, space="PSUM") as episum,
        ):
            # one idx transfer: per-call chunking costs ~650ns of serial
            # HWDGE issue per chunk and the scheduler hoists 8 calls'
            # chunk-waits ahead of the first gather (18us startup); a
            # single 0.7MB DMA lands by ~9us.
            # scalar (ACT) HWDGE ring: does not queue behind the wt
            # prefetches on the SP ring, so the first gather's idx
            # dependency lands ~8us earlier.
            idxt = persist.tile([128, ncols * 8], dt.int16)
            nc.scalar.dma_start(out=idxt[:], in_=idx_d[:, :])

            wot = persist.tile([128, D], dt.bfloat16)
            nc.sync.dma_start(out=wot[:], in_=wo_d[:, :])

            zeroL = persist.tile([128, 128], dt.bfloat16)
            zeroR = persist.tile([128, 512], dt.bfloat16)
            nc.vector.memset(zeroL[:], 0.0)
            nc.vector.memset(zeroR[:], 0.0)

            accbanks = [psump.tile([128, 512], dt.float32, tag=f"accb{i}",
                                   name=f"accb{i}") for i in range(nbank)]
            for bank in accbanks:
                nc.tensor.matmul(out=bank[:, :], lhsT=zeroL[:], rhs=zeroR[:],
                                 start=True, stop=False, skip_group_check=True)

            for h, (c0, cw) in enumerate(calls):
                w = c0 // CS
                nslots = cw * 128

                g = big.tile([128, cw * 2 * D], dt.bfloat16, tag="g")
                nc.gpsimd.dma_gather(
                    out_ap=g[:].rearrange("p (c d) -> p c d", d=2 * D),
                    in_ap=featp_d[w * WPAIRS:PAIRS, :],
                    idxs_ap=idxt[:, c0 * 8:(c0 + cw) * 8],
                    num_idxs=nslots,
                    num_idxs_reg=nslots,
                    elem_size=2 * D,
                    single_packet=False,
                    queue_num=h % 4,
                )

                wtt = wtp.tile([128, cw * 2 * GCAP], dt.bfloat16, tag="wt")
                nc.sync.dma_start(
                    out=wtt[:],
                    in_=wt_d[:, c0 * 2 * GCAP:(c0 + cw) * 2 * GCAP])

                lastw = (w == NW - 1)
                for j in range(cw):
                    grp = c0 - w * CS + j
                    bank, off = grp // 8, (grp % 8) * GCAP
                    nc.tensor.matmul(
                        out=accbanks[bank][:, off:off + GCAP],
                        lhsT=g[:, (2 * j) * D:(2 * j + 1) * D],
                        rhs=wtt[:, (2 * j) * GCAP:(2 * j + 1) * GCAP],
                        start=False, stop=False, skip_group_check=True,
                    )
                    nc.tensor.matmul(
                        out=accbanks[bank][:, off:off + GCAP],
                        lhsT=g[:, (2 * j + 1) * D:(2 * j + 2) * D],
                        rhs=wtt[:, (2 * j + 1) * GCAP:(2 * j + 2) * GCAP],
                        start=False, stop=lastw, skip_group_check=True,
                    )

            for bank in range(nbank):
                ngrp_b = min(ngrp - bank * 8, 8)
                ncol_b = ngrp_b * GCAP
                nj = (ncol_b + 127) // 128
                asb = epi.tile([128, 512], dt.bfloat16, tag="asb")
                nc.vector.tensor_copy(out=asb[:, :ncol_b],
                                      in_=accbanks[bank][:, :ncol_b])
                pso = episum.tile([128, 512], dt.float32, tag="eps")
                for j in range(nj):
                    w_j = min(128, ncol_b - j * 128)
                    nc.tensor.matmul(out=pso[:w_j, j * D:(j + 1) * D],
                                     lhsT=asb[:, j * 128:j * 128 + w_j],
                                     rhs=wot[:],
                                     start=(j == 0), stop=(j == nj - 1),
                                     skip_group_check=True)
                osb = epi.tile([128, nj * D], dt.bfloat16, tag="osb")
                nc.vector.tensor_copy(out=osb[:], in_=pso[:, :nj * D])
                nc.scalar.dma_start(
                    out=o_d[bank * 128:(bank + 1) * 128, :nj * D],
                    in_=osb[:],
                )

    nc.compile()
    return nc


def _host_prep(features, node, neighbours, kernel, kernel1, attention_weights,
               neigh_weights):
    a1 = attention_weights[0, :D]
    a2 = attention_weights[0, D:]
    w1 = kernel1[0] @ a1
    w2 = kernel[0] @ a2
    wout = np.ascontiguousarray(kernel1[0] @ neigh_weights, np.float32)

    sigma = features @ w1
    nt = features[node[:, 0]] @ w2
    score = sigma[neighbours] + nt[:, None]
    lr = np.where(score > 0, score, 0.2 * score)
    m = lr.max(axis=1, keepdims=True)
    e = np.exp(lr - m)
    wgt = (e / e.sum(axis=1, keepdims=True)).astype(np.float32)
    return wout, wgt


def kernel(features, node, neighbours, kernel, kernel1, attention_weights,
           neigh_weights):
    from concourse.bass_utils import run_bass_kernel_spmd

    features = np.asarray(features, np.float32)
    node = np.asarray(node, np.int32)
    neighbours = np.asarray(neighbours, np.int32)
    kernel = np.asarray(kernel, np.float32)
    kernel1 = np.asarray(kernel1, np.float32)
    attention_weights = np.asarray(attention_weights, np.float32)
    neigh_weights = np.asarray(neigh_weights, np.float32)

    wout, wgt = _host_prep(features, node, neighbours, kernel, kernel1,
                           attention_weights, neigh_weights)
    featb_all = features.astype(bf16)
    woutb = wout.astype(bf16)
    rng = np.random.default_rng(0)

    for ngrp in NGRP_TRY:
        prep = []
        ok = True
        for c in range(NCORES):
            nb = neighbours[c * BPC:(c + 1) * BPC]
            wg = wgt[c * BPC:(c + 1) * BPC]
            t = None
            for salt in range(6):
                t, rmap = _prepare_core(nb, wg, ngrp, salt)
                if t is not None:
                    break
            if t is None:
                ok = False
                break
            prep.append((t, rmap))
        if ok:
            break
    else:
        raise RuntimeError("window packing failed at all NGRP values")

    in_maps = [{
        "featp": featb_all[t["perm"]].reshape(PAIRS, 2 * D),
        "idx16": t["idx16"],
        "wt": t["wt"],
        "Wout": woutb,
    } for t, _ in prep]
    row_maps = [rmap for _, rmap in prep]

    key = f"v5fnc{ngrp}"
    if key not in _CACHE:
        _CACHE[key] = _build_program(ngrp)
    nc = _CACHE[key]

    res = run_bass_kernel_spmd(nc, in_maps, core_ids=list(range(NCORES)))
    _CACHE["last_results"] = res
    out = np.zeros((B, D), np.float32)
    for c in range(NCORES):
        ocb = np.asarray(res.results[c]["o"], dtype=np.float32)
        nbank = (ngrp + 7) // 8
        oc = np.zeros((ngrp * GCAP, D), np.float32)
        for b in range(nbank):
            ngrp_b = min(ngrp - b * 8, 8)
            nj = (ngrp_b * GCAP + 127) // 128
            blk = ocb[b * 128:(b + 1) * 128, :nj * D].reshape(128, nj, D)
            oc[b * 512:b * 512 + nj * 128] = blk.transpose(1, 0, 2).reshape(
                nj * 128, D)
        rmap = row_maps[c]
        valid = rmap >= 0
        out[c * BPC + rmap[valid]] = oc[valid]
    return out
